# revision 31
# baseline (speedup 1.0000x reference)
"""MetaBaseline (retrieval_knn) Trainium2 kernel, v3 (bf16).

Computation (per episode b):
  q  = l2norm(input1[b])            # [75, 25, 640] over channel
  s  = l2norm(input2[b])            # [5, 5, 25, 640]
  att = softmax_hw(s @ rpn_w)       # rpn_b is softmax-invariant
  cg  = leaky(sum_hw(att * s))
  feat = mean_shot(mean_hw(s) + 5 * cg)
  sim[b] = mean_hw(q) @ feat.T      # [75, 5]

Sharding: data-parallel over episodes, 4 per core on 8 cores.

Design (HBM-read bound at ~176 GB/s/core with all 8 cores reading):
- bf16 inputs (host-cast): halves HBM traffic vs f32. rel err ~4e-3.
- All 4 episodes resident in SBUF. Bulk loads go on the gpsimd SWDGE
  queue (sprays each transfer across all 16 SDMA engines; the sync
  HWDGE path pins one engine per transfer, ~26 GB/s). Emission order ==
  arrival order: s0, (masks), s1..s3, q0a..q3b.
- Descriptors on partitions (125/tile), channels free. All group
  reductions (hw-mean, softmax Z, shot-mean) are PE matmuls against
  block-mask stationaries with per-descriptor weights folded in.
- Support side fused: per slot one [125, 50] stationary (ut|mt) produces
  cg rows 0-24 and hw-mean rows 25-49 in one pass over s; evac keeps
  partition alignment (lk rows 0-24, sm rows 25-49 of g50) and the
  shot-mean matmul uses shotm50 (top diag 1.0 = GAMMA/SHOT, bottom diag
  1/SHOT; bottom half placed by a one-time SBUF->SBUF DMA) to emit
  feat^T chunks directly -- no feat transposes.
- 1/sqrt via ACT table math (Exp(-0.5*Ln(x))), freeing the DVE.
- leaky(cg/Z) in one ACT Prelu with per-partition scale=1/Z.
- Outputs go out on the scalar (ACT) HWDGE queue.
"""

import os
import sys
from contextlib import ExitStack

sys.path.insert(0, "/opt/trn_rl_repo")

import ml_dtypes
import numpy as np

import concourse.bass as bass
import concourse.tile as tile
from concourse import bacc, mybir
from concourse.bass_utils import run_bass_kernel_spmd

F32 = mybir.dt.float32
BF = mybir.dt.bfloat16
OP = mybir.AluOpType
AF = mybir.ActivationFunctionType

# Problem constants (fixed by the problem statement).
B, QN, WAY, SHOT, HH, WW, C = 32, 75, 5, 5, 5, 5, 640
NCORES = 8
E = B // NCORES        # 4 episodes per core
HW = HH * WW           # 25 spatial positions
P = 125                # descriptors per partition-tile
QT = 15                # query slots per partition row (1875 = 125*15)
ST = 5                 # support slots per partition row (625 = 125*5)
NMAP = WAY * SHOT      # 25 support maps / episode
GAMMA = 5.0
ALPHA = 0.01
CA, CB = 512, 128      # channel chunks (psum bank = 512 f32)
HALF = 8 * C           # q slots 0-7 in the first DMA half
# CoreSim lacks Prelu; substitute Relu when simulating (debug only)
_SIM_SAFE = bool(os.environ.get("KERNEL_SIM_SAFE"))


def _build_body(ctx: ExitStack, tc: "tile.TileContext", i1, i2, rpnw, out):
    nc = tc.nc

    const_pool = ctx.enter_context(tc.tile_pool(name="const", bufs=1))
    data_pool = ctx.enter_context(tc.tile_pool(name="data", bufs=1))
    scr_pool = ctx.enter_context(tc.tile_pool(name="scratch", bufs=1))
    stats = ctx.enter_context(tc.tile_pool(name="stats", bufs=2))
    sel_pool = ctx.enter_context(tc.tile_pool(name="sel", bufs=2))
    sb_pool = ctx.enter_context(tc.tile_pool(name="sbwork", bufs=2))

    ps_a = ctx.enter_context(tc.tile_pool(name="psA", bufs=2, space="PSUM"))
    ps_b = ctx.enter_context(tc.tile_pool(name="psB", bufs=2, space="PSUM"))
    ps_s = ctx.enter_context(tc.tile_pool(name="psS", bufs=3, space="PSUM"))
    ps_d = ctx.enter_context(tc.tile_pool(name="psD", bufs=1, space="PSUM"))

    # Pin the ACT table set to 6 = natural_log_exp_and_others: it contains
    # every function used here (Square/Exp/Ln/Prelu/Copy), so the compiler's
    # per-function greedy set selection never needs to thrash table loads.
    nc.scalar.add_instruction(mybir.InstLoadActFuncSet(
        name=nc.scalar.bass.get_next_instruction_name(),
        ins=[], outs=[], act_func_set_id=6))

    # ---- rpn_w on the (otherwise idle) sync HWDGE queue: arrives ~1us
    w_sb = const_pool.tile([1, C], F32, name="w_sb")
    nc.sync.dma_start(w_sb[:], rpnw)

    # ---- first support episode on the SWDGE queue immediately
    sbig = [data_pool.tile([P, ST * C], BF, name=f"s_{e}") for e in range(E)]
    nc.gpsimd.dma_start(sbig[0][:], i2[0])

    # ---- constants (Pool engine; overlaps the s0 transfer) ----
    # qmcat[p, j, q] = 1/25 iff descriptor 15p+j belongs to query q
    qmcat = const_pool.tile([P, QT, QN], BF, name="qmcat")
    nc.gpsimd.memset(qmcat[:], 1.0 / HW)
    nc.gpsimd.affine_select(
        out=qmcat[:], in_=qmcat[:], pattern=[[1, QT], [-HW, QN]],
        compare_op=OP.is_ge, fill=0.0, base=0, channel_multiplier=QT)
    nc.gpsimd.affine_select(
        out=qmcat[:], in_=qmcat[:], pattern=[[-1, QT], [HW, QN]],
        compare_op=OP.is_ge, fill=0.0, base=HW - 1, channel_multiplier=-QT)
    # smcat[p, j, r, m] = 1 iff descriptor 5p+j belongs to map m (r = ut|mt).
    # The map axis is padded to 32 so the mt block lands on PSUM rows 32-56
    # (PSUM reads must be 32-partition aligned); the staircase bounds zero
    # the pad columns automatically.
    MW = 32
    smcat = const_pool.tile([P, ST, 2, MW], BF, name="smcat")
    nc.gpsimd.memset(smcat[:], 1.0)
    nc.gpsimd.affine_select(
        out=smcat[:], in_=smcat[:], pattern=[[1, ST], [0, 2], [-HW, MW]],
        compare_op=OP.is_ge, fill=0.0, base=0, channel_multiplier=ST)
    nc.gpsimd.affine_select(
        out=smcat[:], in_=smcat[:], pattern=[[-1, ST], [0, 2], [HW, MW]],
        compare_op=OP.is_ge, fill=0.0, base=HW - 1, channel_multiplier=-ST)
    # shot-mean block diagonals: hi part (leaky-cg, GAMMA/SHOT) at rows
    # 0-24; lo part (hw-mean, 1/SHOT) DMA'd to rows 32-56 of a 64-row tile
    # (partition-shifting copies are DMA-only).
    shotm_hi = const_pool.tile([NMAP, WAY], BF, name="shotm_hi")
    nc.gpsimd.memset(shotm_hi[:], GAMMA / SHOT)
    nc.gpsimd.affine_select(
        out=shotm_hi[:], in_=shotm_hi[:], pattern=[[-SHOT, WAY]],
        compare_op=OP.is_ge, fill=0.0, base=0, channel_multiplier=1)
    nc.gpsimd.affine_select(
        out=shotm_hi[:], in_=shotm_hi[:], pattern=[[SHOT, WAY]],
        compare_op=OP.is_ge, fill=0.0, base=SHOT - 1, channel_multiplier=-1)
    shotm_lo = const_pool.tile([NMAP, WAY], BF, name="shotm_lo")
    nc.gpsimd.memset(shotm_lo[:], 1.0 / SHOT)
    nc.gpsimd.affine_select(
        out=shotm_lo[:], in_=shotm_lo[:], pattern=[[-SHOT, WAY]],
        compare_op=OP.is_ge, fill=0.0, base=0, channel_multiplier=1)
    nc.gpsimd.affine_select(
        out=shotm_lo[:], in_=shotm_lo[:], pattern=[[SHOT, WAY]],
        compare_op=OP.is_ge, fill=0.0, base=SHOT - 1, channel_multiplier=-1)
    shotm_lo32 = const_pool.tile([MW + NMAP, WAY], BF, name="shotm_lo32")
    nc.gpsimd.dma_start(shotm_lo32[MW:MW + NMAP, :], shotm_lo[:])
    # identity (transposes)
    ident = const_pool.tile([128, 128], BF, name="ident")
    nc.gpsimd.memset(ident[:], 1.0)
    nc.gpsimd.affine_select(
        out=ident[:], in_=ident[:], pattern=[[-1, 128]],
        compare_op=OP.is_equal, fill=0.0, base=0, channel_multiplier=1)

    # ---- the rest of the bulk loads, in arrival order, on SWDGE ----
    for e in range(1, E):
        nc.gpsimd.dma_start(sbig[e][:], i2[e])
    qbig = [data_pool.tile([P, QT * C], BF, name=f"q_{e}") for e in range(E)]
    for e in range(E):
        nc.gpsimd.dma_start(qbig[e][:, 0:HALF], i1[e, :, 0:HALF])
        nc.gpsimd.dma_start(qbig[e][:, HALF:QT * C], i1[e, :, HALF:QT * C])

    # rpn_w broadcast to all partitions via a k=1 matmul (bf16)
    w_bf = const_pool.tile([1, C], BF, name="w_bf")
    nc.scalar.copy(w_bf[:], w_sb[:])
    ones1 = const_pool.tile([1, P], BF, name="ones1")
    nc.vector.memset(ones1[:], 1.0)
    wb = const_pool.tile([P, C], BF, name="wb")
    wb_psA = ps_a.tile([P, CA], F32, name="wbA", tag="psA")
    nc.tensor.matmul(wb_psA[:], ones1[:], w_bf[:, 0:CA])
    nc.scalar.copy(wb[:, 0:CA], wb_psA[:])
    wb_psB = ps_b.tile([P, CB], F32, name="wbB", tag="psB")
    nc.tensor.matmul(wb_psB[:], ones1[:], w_bf[:, CA:C])
    nc.scalar.copy(wb[:, CA:C], wb_psB[:])

    def keep_warm(rhs_tile, slices):
        """Discarded matmuls that become ready the moment their DMA lands:
        they fill the PE's idle window while ACT/DVE compute norms, keeping
        the HAM clock gate at 2.4 GHz for the real matmuls that follow
        (PE re-throttles to 1.2 GHz after ~3.4us idle)."""
        dum = ps_d.tile([64, CA], F32, name="dum", tag="dum")
        for c0 in slices:
            nc.tensor.matmul(dum[:], qmcat[:, 0, 0:64], rhs_tile[:, c0:c0 + CA])

    keep_warm(sbig[0], [512 * k for k in range(4)])

    # featT for all episodes, written per-episode: [128 c, 5 chunks x 20]
    ftT4 = data_pool.tile([128, 5 * E * WAY], BF, name="ftT4")
    EW = E * WAY

    I32 = mybir.dt.int32

    def newton_rsqrt(dst, x, n, tag):
        """dst = x^-0.5 on ACT: Exp(-0.5 * Ln(x)). Both live in act-table
        set 6 (natural_log_exp_and_others), pinned once at program start.
        DVE-based rsqrt variants (bit-trick or reciprocal+Newton) fault
        this kernel on hardware; ACT table math is the stable path."""
        t = stats.tile([P, n], F32, name=f"nw_t_{tag}", tag=f"nw_t_{tag}")
        nc.scalar.activation(t[:], x, AF.Ln)
        nc.scalar.activation(dst, t[:], AF.Exp, scale=-0.5)

    # ================= support phase (per episode) =================
    for e in range(E):
        sn2 = stats.tile([P, ST], F32, name=f"sn2_{e}", tag="sn2")
        rr = stats.tile([P, ST], F32, name=f"rr_{e}", tag="rr")
        for j in range(ST):
            sl = sbig[e][:, C * j:C * (j + 1)]
            if j < 2:
                scr = scr_pool.tile([P, C], BF, name="sqa", tag="sqa")
                nc.scalar.activation(scr[:], sl, AF.Square,
                                     accum_out=sn2[:, j:j + 1])
            else:
                scr = scr_pool.tile([P, C], BF, name="sqv", tag="sqv")
                nc.vector.scalar_tensor_tensor(
                    out=scr[:], in0=sl, scalar=1.0, in1=sl,
                    op0=OP.mult, op1=OP.mult, accum_out=sn2[:, j:j + 1])
            scr2 = scr_pool.tile([P, C], BF, name="lgscr", tag="lgscr")
            nc.vector.scalar_tensor_tensor(
                out=scr2[:], in0=sl, scalar=1.0, in1=wb[:],
                op0=OP.mult, op1=OP.mult, accum_out=rr[:, j:j + 1])
        sinv = stats.tile([P, ST], F32, name=f"sinv_{e}", tag="sinv")
        newton_rsqrt(sinv[:], sn2[:], ST, f"s{e % 2}")
        # softmax over hw within each map (logits tiny: no max-shift)
        lg = stats.tile([P, ST], F32, name=f"lg_{e}", tag="lg")
        nc.vector.tensor_mul(lg[:], rr[:], sinv[:])
        el = stats.tile([P, ST], F32, name=f"el_{e}", tag="el")
        nc.scalar.activation(el[:], lg[:], AF.Exp)
        el_bf = stats.tile([P, ST], BF, name=f"elb_{e}", tag="elb")
        nc.vector.tensor_copy(el_bf[:], el[:])
        # per-map Z = sum of exp (PE partition-reduce, 5 tiny chained MMs)
        sums = ps_s.tile([NMAP, 1], F32, name=f"sums_{e}", tag="ps_small")
        for j in range(ST):
            nc.tensor.matmul(sums[:], smcat[:, j, 0, 0:NMAP],
                             el_bf[:, j:j + 1],
                             start=(j == 0), stop=(j == ST - 1))
        rec = stats.tile([NMAP, 1], F32, name=f"rec_{e}", tag="rec")
        nc.vector.reciprocal(rec[:], sums[:])
        # weights: ut = e^l * inv (unnormalized att*inv), mt = inv/25
        wcat = stats.tile([P, ST, 2], BF, name=f"wcat_{e}", tag="wcat")
        nc.vector.tensor_mul(wcat[:, :, 0], el[:], sinv[:])
        nc.vector.tensor_scalar_mul(wcat[:, :, 1], sinv[:], 1.0 / HW)
        big_l = sel_pool.tile([P, ST, 2, MW], BF, name="big_l", tag="big_l")
        nc.vector.scalar_tensor_tensor(
            out=big_l[:], in0=smcat[:], scalar=1.0,
            in1=wcat[:].unsqueeze(3).broadcast_to([P, ST, 2, MW]),
            op0=OP.mult, op1=OP.mult)
        # fused cg|sm: rows 0-24 = cg (unnormalized), rows 32-56 = mean_hw
        spsA = ps_a.tile([2 * MW, CA], F32, name=f"sA_{e}", tag="psA")
        spsB = ps_b.tile([2 * MW, CB], F32, name=f"sB_{e}", tag="psB")
        for j in range(ST):
            sl = sbig[e][:, C * j:C * (j + 1)]
            nc.tensor.matmul(spsA[:], big_l[:, j], sl[:, 0:CA],
                             start=(j == 0), stop=(j == ST - 1))
            nc.tensor.matmul(spsB[:], big_l[:, j], sl[:, CA:C],
                             start=(j == 0), stop=(j == ST - 1))
        # g64 rows 0-24 = leaky(cg/Z) (ACT Prelu, scale=1/Z);
        # rows 32-56 = sm (plain copies; PSUM base 32 is aligned)
        g64 = sb_pool.tile([2 * MW, C], BF, name="g64", tag="g64")
        _lrelu = AF.Relu if _SIM_SAFE else AF.Prelu
        nc.scalar.activation(g64[0:NMAP, 0:CA], spsA[0:NMAP, :], _lrelu,
                             scale=rec[:, 0:1], alpha=ALPHA)
        nc.scalar.activation(g64[0:NMAP, CA:C], spsB[0:NMAP, :], _lrelu,
                             scale=rec[:, 0:1], alpha=ALPHA)
        nc.vector.tensor_copy(g64[MW:MW + NMAP, 0:CA],
                              spsA[MW:MW + NMAP, :])
        nc.vector.tensor_copy(g64[MW:MW + NMAP, CA:C],
                              spsB[MW:MW + NMAP, :])
        # featT directly: two chained MMs (lk part + sm part) per c-chunk
        for cc in range(5):
            tp = ps_s.tile([128, WAY], F32, name="ftT_ps", tag="ps_small")
            nc.tensor.matmul(tp[:], g64[0:NMAP, 128 * cc:128 * (cc + 1)],
                             shotm_hi[:], start=True, stop=False)
            nc.tensor.matmul(tp[:],
                             g64[MW:MW + NMAP, 128 * cc:128 * (cc + 1)],
                             shotm_lo32[MW:MW + NMAP, :],
                             start=False, stop=True)
            nc.vector.tensor_copy(
                ftT4[:, EW * cc + WAY * e:EW * cc + WAY * (e + 1)], tp[:])

    # ================= query phase (per episode) =================
    for e in range(E):
        keep_warm(qbig[e], [512 * k for k in range(7)])
        keep_warm(qbig[e], [HALF + 512 * k for k in range(5)])
        qn2 = stats.tile([P, QT], F32, name=f"qn2_{e}", tag="qn2")
        for j in range(QT):
            sl = qbig[e][:, C * j:C * (j + 1)]
            if j % 5 < 3:      # ACT 9 : DVE 6 per episode
                scr = scr_pool.tile([P, C], BF, name="sqa", tag="sqa")
                nc.scalar.activation(scr[:], sl, AF.Square,
                                     accum_out=qn2[:, j:j + 1])
            else:
                scr = scr_pool.tile([P, C], BF, name="sqv", tag="sqv")
                nc.vector.scalar_tensor_tensor(
                    out=scr[:], in0=sl, scalar=1.0, in1=sl,
                    op0=OP.mult, op1=OP.mult, accum_out=qn2[:, j:j + 1])
        qinv = stats.tile([P, QT], F32, name=f"qinv_{e}", tag="qinv")
        qinv_bf = stats.tile([P, QT], BF, name=f"qinvb_{e}", tag="qinvb")
        sel = sel_pool.tile([P, QT, QN], BF, name="sel", tag="sel")
        # last episode: two halves so slots 0-7 unblock their matmuls early
        splits = ((0, 8), (8, QT)) if e == E - 1 else ((0, QT),)
        for si, (a, b) in enumerate(splits):
            newton_rsqrt(qinv[:, a:b], qn2[:, a:b], b - a, f"q{si}_{e % 2}")
            nc.vector.tensor_copy(qinv_bf[:, a:b], qinv[:, a:b])
            nc.vector.scalar_tensor_tensor(
                out=sel[:, a:b], in0=qmcat[:, a:b], scalar=1.0,
                in1=qinv_bf[:, a:b].unsqueeze(2).broadcast_to([P, b - a, QN]),
                op0=OP.mult, op1=OP.mult)
        qmA = ps_a.tile([QN, CA], F32, name=f"qA_{e}", tag="psA")
        qmB = ps_b.tile([QN, CB], F32, name=f"qB_{e}", tag="psB")
        for j in range(QT):
            sl = qbig[e][:, C * j:C * (j + 1)]
            nc.tensor.matmul(qmA[:], sel[:, j], sl[:, 0:CA],
                             start=(j == 0), stop=(j == QT - 1))
            nc.tensor.matmul(qmB[:], sel[:, j], sl[:, CA:C],
                             start=(j == 0), stop=(j == QT - 1))
        qm_sb = sb_pool.tile([QN, C], BF, name="qm_sb", tag="qm_sb")
        nc.scalar.copy(qm_sb[:, 0:CA], qmA[:])
        nc.scalar.copy(qm_sb[:, CA:C], qmB[:])
        # sim = qm @ feat.T via c-on-partition chunks
        qmT = sb_pool.tile([128, 5 * QN], BF, name="qmT", tag="qmT")
        for cc in range(5):
            tp = ps_s.tile([128, QN], BF, name="qmT_ps", tag="ps_small")
            nc.tensor.transpose(tp[:], qm_sb[:, 128 * cc:128 * (cc + 1)],
                                ident[0:QN, 0:QN])
            nc.vector.tensor_copy(qmT[:, QN * cc:QN * (cc + 1)], tp[:])
        simps = ps_s.tile([QN, WAY], F32, name=f"sim_{e}", tag="ps_small")
        for cc in range(5):
            nc.tensor.matmul(simps[:], qmT[:, QN * cc:QN * (cc + 1)],
                             ftT4[:, EW * cc + WAY * e:EW * cc + WAY * (e + 1)],
                             start=(cc == 0), stop=(cc == 4))
        sim_sb = sb_pool.tile([QN, WAY], F32, name=f"sim_sb_{e}", tag="sim_sb")
        nc.vector.tensor_copy(sim_sb[:], simps[:])
        # output on the ACT HWDGE queue (independent of the input-load FIFO)
        nc.scalar.dma_start(out[e], sim_sb[:])


def build_program():
    nc = bacc.Bacc("TRN2", target_bir_lowering=False, debug=False,
                   num_devices=NCORES)
    inp1 = nc.dram_tensor("input1", [E, P, QT * C], BF, kind="ExternalInput")
    inp2 = nc.dram_tensor("input2", [E, P, ST * C], BF, kind="ExternalInput")
    rpnw = nc.dram_tensor("rpn_w", [1, C], F32, kind="ExternalInput")
    out = nc.dram_tensor("sim", [E, QN, WAY], F32, kind="ExternalOutput")
    with tile.TileContext(nc) as tc, ExitStack() as ctx:
        _build_body(ctx, tc, inp1.ap(), inp2.ap(), rpnw.ap(), out.ap())
    nc.compile()
    return nc


_NC = None


def _get_nc():
    global _NC
    if _NC is None:
        _NC = build_program()
    return _NC


def shard_inputs(input1, input2, rpn_w, rpn_b=None):
    """Shard over episodes; relayout to descriptor-slot form and cast bf16.
    [E, 1875, 640] -> [E, 125, 15*640] is a pure reshape (d = 15p + j)."""
    bf = ml_dtypes.bfloat16
    i1 = np.ascontiguousarray(
        np.asarray(input1, dtype=np.float32).reshape(B, P, QT * C)).astype(bf)
    i2 = np.ascontiguousarray(
        np.asarray(input2, dtype=np.float32).reshape(B, P, ST * C)).astype(bf)
    w = np.ascontiguousarray(np.asarray(rpn_w, dtype=np.float32)).reshape(1, C)
    in_maps = []
    for i in range(NCORES):
        in_maps.append({
            "input1": np.ascontiguousarray(i1[E * i:E * (i + 1)]),
            "input2": np.ascontiguousarray(i2[E * i:E * (i + 1)]),
            "rpn_w": w,
        })
    return in_maps


def _ensure_ntff_hook():
    """Install the NTFF profile hook (the image's antenv lacks axon_hooks)."""
    import types
    import antenv

    if "antenv.axon_hooks" not in sys.modules:
        mod = types.ModuleType("antenv.axon_hooks")
        mod._hook = None
        mod.set_axon_ntff_profile_hook = lambda h: setattr(mod, "_hook", h)
        mod.get_axon_ntff_profile_hook = lambda: mod._hook
        sys.modules["antenv.axon_hooks"] = mod
        antenv.axon_hooks = mod
    mod = sys.modules["antenv.axon_hooks"]
    if mod.get_axon_ntff_profile_hook() is None:
        from trn_agent_boot.trn_boot import _ntff_profile_via_ctypes
        hook = _ntff_profile_via_ctypes("/opt/axon/libaxon_pjrt.so")
        if hook is not None:
            mod.set_axon_ntff_profile_hook(hook)


def kernel(input1, input2, rpn_w, rpn_b=None, **run_kwargs):
    if run_kwargs.get("trace"):
        _ensure_ntff_hook()
    nc = _get_nc()
    in_maps = shard_inputs(input1, input2, rpn_w)
    res = run_bass_kernel_spmd(nc, in_maps, list(range(NCORES)), **run_kwargs)
    out = np.concatenate([r["sim"] for r in res.results], axis=0)
    if run_kwargs:
        kernel.last_results = res
    return out.astype(np.float32)
